# revision 3
# baseline (speedup 1.0000x reference)
"""Trainium2 Bass kernel for nn_AttentionOutput (complex causal leaky-relu attention).

Reference (B=4, N=4096, F=64), per batch:
    sr = (Qr@Kr^T - Qi@Ki^T)/sqrt(N); si = (Qr@Ki^T + Qi@Kr^T)/sqrt(N)
    wr = tril * leaky_relu(sr);        wi = tril * leaky_relu(si)
    out_r = (wr@Vr)@W_att^T + b;       out_i = (wi@Vi)@W_att^T + b

Distribution: 2 cores per batch.  Core parity h processes j-blocks J === h
(mod 2) for ALL 4096 query rows; causal work is then identical across cores
(slot I needs 2I+2 j-blocks), so a single SPMD program serves all 8 cores and
the host sums the two partial outputs per batch.

Host-side layout prep removes every on-device transpose:
  - scores contract over p = f*2+c (128 partitions, ONE matmul per component):
    sr = Qmodr . K^T where Qmodr = Q with odd columns negated, and
    si = Qmodi . K^T where Qmodi = Q with column pairs swapped; K stays plain.
    Both Q variants are fed pre-transposed [128, N].
  - V' = (1/64) V @ W_att^T folds the score scale and the output projection
    into the attention-value matmul (leaky_relu is positively homogeneous).
  - output is stored transposed ([128, N]: y_r^T on rows 0:64, y_i^T on
    64:128); the host untransposes, interleaves, adds bias, sums parities.

leaky_relu lowering (RELU_CORR): leaky(s) = 0.99*relu(s) + 0.01*s.  For
causally-full j-blocks the 0.01*s term telescopes into a per-slot constant
matmul accumulated into the y PSUM bank (weights mcr/mci precomputed on the
host).  Diagonal tiles compute u = mask*s (drain), w = relu(u), and feed two
matmuls against 0.99*V' and 0.01*V'.

v2 pipeline (PE-throughput oriented):
  - flat (slot, j-block) sequence; score matmuls run TWO blocks ahead of the
    value matmuls so PSUM drains are never on the PE critical path.
  - s_r/s_i share the kp stationary: the s_i matmul is emitted with
    ldweights=False so the PE skips the redundant LDWEIGHTS.
  - y_r/y_i packed into ONE PSUM bank (partitions 0:64 / 64:128) via the PE
    column-tiling (out.base_partition=64 for the i half): halves y banks and
    output copies/DMAs.  6 score banks + 2 y banks = all 8 PSUM banks.
  - drains spread over three engines: full tiles 2:1 ACT:DVE, diagonal
    mask-mults on DVE, diagonal relus on GpSimd (SBUF->SBUF).
  - input DMA issued in slot-consumption order; output stored bf16.

NOTE: ACT Lrelu reading PSUM hangs TRN2 (empirically) — never emit it.
"""

import numpy as np

import concourse.bacc as bacc
import concourse.tile as tile
from concourse import mybir
from concourse.bass_utils import run_bass_kernel_spmd

B, N, F = 4, 4096, 64
P = 128             # = 2*F: score contraction width / partition count
JB = 128            # j-block width
IBW = 512           # i-block (slot) width
NSLOT = N // IBW    # 8 slots
NJPAR = N // JB // 2  # 16 parity j-blocks per core
NEG = 0.01
SCALE = 1.0 / 64.0  # 1/sqrt(N)
NCORES = 8

_DT = mybir.dt.float32
MM_BF16 = True      # bf16 matmul inputs: full PE stream rate, half the DMA
SKIP_LDW = True     # s_i reuses the kp stationary loaded by s_r
_CACHE: dict = {}


def _build_nc():
    nc = bacc.Bacc("TRN2", target_bir_lowering=False, num_devices=NCORES)
    dt = _DT
    mdt = mybir.dt.bfloat16 if MM_BF16 else _DT  # matmul input dtype
    qrT = nc.dram_tensor("qrT", [P, N], mdt, kind="ExternalInput")
    qiT = nc.dram_tensor("qiT", [P, N], mdt, kind="ExternalInput")
    kp = nc.dram_tensor("kp", [P, NJPAR * JB], mdt, kind="ExternalInput")
    # va = 0.99 * V' (relu term), vb = 0.01 * V' (raw term, diagonal only)
    var_ = nc.dram_tensor("var", [P, NJPAR * F], mdt, kind="ExternalInput")
    vai = nc.dram_tensor("vai", [P, NJPAR * F], mdt, kind="ExternalInput")
    vbr = nc.dram_tensor("vbr", [P, NJPAR * F], mdt, kind="ExternalInput")
    vbi = nc.dram_tensor("vbi", [P, NJPAR * F], mdt, kind="ExternalInput")
    # per-slot correction weights: 0.01 * sum_{full J} kp_J @ V'_J  [P, 64]
    mcr = nc.dram_tensor("mcr", [P, NSLOT * F], mdt, kind="ExternalInput")
    mci = nc.dram_tensor("mci", [P, NSLOT * F], mdt, kind="ExternalInput")
    dmask = nc.dram_tensor("dmask", [2, JB, IBW], mdt, kind="ExternalInput")
    out = nc.dram_tensor("out", [P, N], mdt, kind="ExternalOutput")

    relu = mybir.ActivationFunctionType.Relu
    mul_op = mybir.AluOpType.mult

    with tile.TileContext(nc) as tc:
        with (
            tc.tile_pool(name="res", bufs=1) as res,
            tc.tile_pool(name="wp", bufs=10) as wp,
            tc.tile_pool(name="osb", bufs=2) as osb,
            tc.tile_pool(name="spsum", bufs=6, space="PSUM") as spsum,
            tc.tile_pool(name="ypsum", bufs=2, space="PSUM") as ypsum,
        ):
            sb_qr = res.tile([P, N], mdt, tag="qr")
            sb_qi = res.tile([P, N], mdt, tag="qi")
            sb_k = res.tile([P, NJPAR * JB], mdt, tag="k")
            sb_var = res.tile([P, NJPAR * F], mdt, tag="var")
            sb_vai = res.tile([P, NJPAR * F], mdt, tag="vai")
            sb_vbr = res.tile([P, NJPAR * F], mdt, tag="vbr")
            sb_vbi = res.tile([P, NJPAR * F], mdt, tag="vbi")
            sb_mcr = res.tile([P, NSLOT * F], mdt, tag="mcr")
            sb_mci = res.tile([P, NSLOT * F], mdt, tag="mci")
            sb_m0 = res.tile([JB, IBW], mdt, tag="m0")
            sb_m1 = res.tile([JB, IBW], mdt, tag="m1")

            def dma(dst, src, c):
                sl = slice(c * 512, (c + 1) * 512)
                nc.sync.dma_start(out=dst[:, sl], in_=src[:, sl])

            # slot-consumption order: slot s needs q chunk s, kp chunk
            # (2s+2)/4, v chunks from slot 4, mcr/mci from slot 1.
            nc.sync.dma_start(out=sb_m0, in_=dmask[0])
            nc.sync.dma_start(out=sb_m1, in_=dmask[1])
            dma(sb_k, kp, 0)
            dma(sb_qr, qrT, 0)
            dma(sb_qi, qiT, 0)
            for t_sb, t_dr in ((sb_var, var_), (sb_vai, vai),
                               (sb_vbr, vbr), (sb_vbi, vbi)):
                dma(t_sb, t_dr, 0)
            nc.sync.dma_start(out=sb_mcr, in_=mcr[:])
            nc.sync.dma_start(out=sb_mci, in_=mci[:])
            dma(sb_qr, qrT, 1)
            dma(sb_qi, qiT, 1)
            dma(sb_k, kp, 1)
            dma(sb_qr, qrT, 2)
            dma(sb_qi, qiT, 2)
            dma(sb_qr, qrT, 3)
            dma(sb_qi, qiT, 3)
            dma(sb_k, kp, 2)
            for t_sb, t_dr in ((sb_var, var_), (sb_vai, vai),
                               (sb_vbr, vbr), (sb_vbi, vbi)):
                dma(t_sb, t_dr, 1)
            dma(sb_qr, qrT, 4)
            dma(sb_qi, qiT, 4)
            dma(sb_k, kp, 3)
            for c in range(5, 8):
                dma(sb_qr, qrT, c)
                dma(sb_qi, qiT, c)

            sb_masks = (sb_m0, sb_m1)
            seq = [(s, p) for s in range(NSLOT) for p in range(2 * s + 2)]
            pend = {}    # idx -> per-comp drained tiles
            ytile = {}   # slot -> packed PSUM bank [P, IBW]
            drain_ctr = 0

            for idx in range(len(seq) + 2):
                if idx < len(seq):
                    s, p = seq[idx]
                    cnt = 2 * s + 2
                    isl = slice(s * IBW, (s + 1) * IBW)
                    if p == 0:
                        y = ytile[s] = ypsum.tile([P, IBW], dt, tag="y",
                                                  name=f"y{s}")
                        if s > 0:
                            msl = slice(s * F, (s + 1) * F)
                            nc.tensor.matmul(y[0:64, :], sb_mcr[:, msl],
                                             sb_qr[:, isl],
                                             start=True, stop=False)
                            nc.tensor.matmul(y[64:128, :], sb_mci[:, msl],
                                             sb_qi[:, isl],
                                             start=True, stop=False)
                    # scores: s_i reuses the kp stationary loaded by s_r
                    ksl = slice(p * JB, (p + 1) * JB)
                    s_r = spsum.tile([JB, IBW], dt, tag="s")
                    nc.tensor.matmul(s_r[:], sb_k[:, ksl], sb_qr[:, isl],
                                     start=True, stop=True)
                    s_i = spsum.tile([JB, IBW], dt, tag="s")
                    mm_i = nc.tensor.matmul(s_i[:], sb_k[:, ksl],
                                            sb_qi[:, isl],
                                            start=True, stop=True)
                    if SKIP_LDW:
                        mm_i.ins.ldweights = False
                    # drains (off the PE critical path; values lag 2 blocks)
                    tiles = []
                    for s_ps in (s_r, s_i):
                        if p < cnt - 2:
                            w = wp.tile([JB, IBW], mdt, tag="w")
                            if drain_ctr % 3 == 2:
                                nc.vector.tensor_scalar_max(w[:], s_ps[:], 0.0)
                            else:
                                nc.scalar.activation(w[:], s_ps[:], relu)
                            drain_ctr += 1
                            tiles.append((w,))
                        else:
                            mk = sb_masks[p - (cnt - 2)]
                            u = wp.tile([JB, IBW], mdt, tag="u")
                            nc.vector.tensor_tensor(out=u[:], in0=s_ps[:],
                                                    in1=mk[:], op=mul_op)
                            w = wp.tile([JB, IBW], mdt, tag="w")
                            nc.gpsimd.tensor_scalar_max(w[:], u[:], 0.0)
                            tiles.append((u, w))
                    pend[idx] = tiles
                if idx >= 2:
                    s2, p2 = seq[idx - 2]
                    cnt2 = 2 * s2 + 2
                    y = ytile[s2]
                    vsl = slice(p2 * F, (p2 + 1) * F)
                    tiles = pend.pop(idx - 2)
                    for comp, (sb_va, sb_vb, psl) in enumerate((
                            (sb_var, sb_vbr, slice(0, 64)),
                            (sb_vai, sb_vbi, slice(64, 128)))):
                        first = (s2 == 0 and p2 == 0)
                        last = (p2 == cnt2 - 1)
                        tl = tiles[comp]
                        if len(tl) == 1:
                            nc.tensor.matmul(y[psl, :], sb_va[:, vsl], tl[0][:],
                                             start=first, stop=False)
                        else:
                            u, w = tl
                            nc.tensor.matmul(y[psl, :], sb_vb[:, vsl], u[:],
                                             start=first, stop=False)
                            nc.tensor.matmul(y[psl, :], sb_va[:, vsl], w[:],
                                             start=False, stop=last)
                    if p2 == cnt2 - 1:
                        isl2 = slice(s2 * IBW, (s2 + 1) * IBW)
                        y_sb = osb.tile([P, IBW], mdt, tag="ysb")
                        nc.scalar.copy(y_sb[:], y[:])
                        nc.sync.dma_start(out=out[:, isl2], in_=y_sb[:])
    nc.compile()
    return nc


def _prep_inputs(Q, K, V, W_att, b_att):
    """Host-side re-layout: per-core in_maps for run_bass_kernel_spmd."""
    Q = np.asarray(Q, dtype=np.float32)
    K = np.asarray(K, dtype=np.float32)
    V = np.asarray(V, dtype=np.float32)
    W_att = np.asarray(W_att, dtype=np.float32)

    Qf = Q.reshape(B, N, P)          # [b, i, f*2+c]
    Kf = K.reshape(B, N, P)
    Vpr = SCALE * (V[..., 0] @ W_att.T)   # [B, N, F]
    Vpi = SCALE * (V[..., 1] @ W_att.T)

    # causal masks for a slot's last two parity j-blocks, per core parity h:
    # diagonal sub-block d = 2k+h of the slot's group of 4
    jj = np.arange(JB)[:, None]
    ii = np.arange(IBW)[None, :]
    masks = {h: np.stack([(ii >= jj + JB * (2 * k + h)).astype(np.float32)
                          for k in range(2)]) for h in (0, 1)}

    if MM_BF16:
        import ml_dtypes
        cvt = lambda a: np.ascontiguousarray(a).astype(ml_dtypes.bfloat16)
    else:
        cvt = lambda a: np.ascontiguousarray(a, dtype=np.float32)

    in_maps = []
    for c in range(NCORES):
        b, h = divmod(c, 2)
        Qmodr = Qf[b].copy()
        Qmodr[:, 1::2] *= -1.0
        Qmodi = np.empty_like(Qf[b])
        Qmodi[:, 0::2] = Qf[b][:, 1::2]
        Qmodi[:, 1::2] = Qf[b][:, 0::2]
        # parity-packed K: [P, NJPAR*JB], position pp holds block J = 2*pp+h
        kp3 = Kf[b].reshape(N // JB, JB, P)[h::2]          # [16, j, p]
        kp = kp3.transpose(2, 0, 1).reshape(P, -1)         # [p, pp*JB+j]
        vr3 = Vpr[b].reshape(N // JB, JB, F)[h::2]         # [16, j, f]
        vi3 = Vpi[b].reshape(N // JB, JB, F)[h::2]
        vpr = vr3.transpose(1, 0, 2).reshape(JB, -1)       # [j, pp*F+f]
        vpi = vi3.transpose(1, 0, 2).reshape(JB, -1)
        # per-slot correction: 0.01 * sum over FULL blocks (pos < cnt-2 = 2s)
        prod_r = np.einsum('bjp,bjf->bpf', kp3, vr3)       # [16, p, f]
        prod_i = np.einsum('bjp,bjf->bpf', kp3, vi3)
        pre_r = np.concatenate(
            [np.zeros((1, P, F), np.float32), np.cumsum(prod_r, axis=0)])
        pre_i = np.concatenate(
            [np.zeros((1, P, F), np.float32), np.cumsum(prod_i, axis=0)])
        mcr = np.concatenate([NEG * pre_r[2 * s] for s in range(NSLOT)], axis=1)
        mci = np.concatenate([NEG * pre_i[2 * s] for s in range(NSLOT)], axis=1)
        in_maps.append({
            "qrT": cvt(Qmodr.T),
            "qiT": cvt(Qmodi.T),
            "kp": cvt(kp),
            "var": cvt((1.0 - NEG) * vpr),
            "vai": cvt((1.0 - NEG) * vpi),
            "vbr": cvt(NEG * vpr),
            "vbi": cvt(NEG * vpi),
            "mcr": cvt(mcr),
            "mci": cvt(mci),
            "dmask": cvt(masks[h]),
        })
    return in_maps


def _gather(results, b_att):
    b_att = np.asarray(b_att, dtype=np.float32)
    out = np.empty((B, N, F, 2), dtype=np.float32)
    for b in range(B):
        y = (results[2 * b]["out"].astype(np.float32)
             + results[2 * b + 1]["out"].astype(np.float32))  # [128, N]
        out[b, :, :, 0] = y[0:64].T + b_att[None, :]
        out[b, :, :, 1] = y[64:128].T + b_att[None, :]
    return out


def kernel(Q, K, V, W_att, b_att):
    if "nc" not in _CACHE:
        _CACHE["nc"] = _build_nc()
    nc = _CACHE["nc"]
    in_maps = _prep_inputs(Q, K, V, W_att, b_att)
    res = run_bass_kernel_spmd(nc, in_maps, core_ids=list(range(NCORES)))
    return _gather(res.results, b_att)


# revision 4
# speedup vs baseline: 3.0660x; 3.0660x over previous
"""Trainium2 Bass kernel for nn_AttentionOutput (complex causal leaky-relu attention).

Reference (B=4, N=4096, F=64), per batch:
    sr = (Qr@Kr^T - Qi@Ki^T)/sqrt(N); si = (Qr@Ki^T + Qi@Kr^T)/sqrt(N)
    wr = tril * leaky_relu(sr);        wi = tril * leaky_relu(si)
    out_r = (wr@Vr)@W_att^T + b;       out_i = (wi@Vi)@W_att^T + b

Distribution: 2 cores per batch.  Core parity h processes j-blocks J === h
(mod 2) for ALL 4096 query rows; causal work is then identical across cores
(slot I needs 2I+2 j-blocks), so a single SPMD program serves all 8 cores and
the host sums the two partial outputs per batch.

Host-side layout prep removes every on-device transpose:
  - scores contract over p = f*2+c (128 partitions, ONE matmul per component):
    sr = Qmodr . K^T where Qmodr = Q with odd columns negated, and
    si = Qmodi . K^T where Qmodi = Q with column pairs swapped; K stays plain.
    Both Q variants are fed pre-transposed [128, N].
  - V' = (1/64) V @ W_att^T folds the score scale and the output projection
    into the attention-value matmul (leaky_relu is positively homogeneous).
  - output is stored transposed ([128, N]: y_r^T on rows 0:64, y_i^T on
    64:128); the host untransposes, interleaves, adds bias, sums parities.

leaky_relu lowering (RELU_CORR): leaky(s) = 0.99*relu(s) + 0.01*s.  For
causally-full j-blocks the 0.01*s term telescopes into a per-slot constant
matmul accumulated into the y PSUM bank (weights mcr/mci precomputed on the
host).  Diagonal tiles compute u = mask*s (drain), w = relu(u), and feed two
matmuls against 0.99*V' and 0.01*V'.

v2 pipeline (PE-throughput oriented):
  - flat (slot, j-block) sequence; score matmuls run TWO blocks ahead of the
    value matmuls so PSUM drains are never on the PE critical path.
  - s_r/s_i share the kp stationary: the s_i matmul is emitted with
    ldweights=False so the PE skips the redundant LDWEIGHTS.
  - y_r/y_i packed into ONE PSUM bank (partitions 0:64 / 64:128) via the PE
    column-tiling (out.base_partition=64 for the i half): halves y banks and
    output copies/DMAs.  6 score banks + 2 y banks = all 8 PSUM banks.
  - drains spread over three engines: full tiles 2:1 ACT:DVE, diagonal
    mask-mults on DVE, diagonal relus on GpSimd (SBUF->SBUF).
  - input DMA issued in slot-consumption order; output stored bf16.

NOTE: ACT Lrelu reading PSUM hangs TRN2 (empirically) — never emit it.
"""

import numpy as np

import concourse.bacc as bacc
import concourse.tile as tile
from concourse import mybir
from concourse.bass_utils import run_bass_kernel_spmd

B, N, F = 4, 4096, 64
P = 128             # = 2*F: score contraction width / partition count
JB = 128            # j-block width
IBW = 512           # i-block (slot) width
NSLOT = N // IBW    # 8 slots
NJPAR = N // JB // 2  # 16 parity j-blocks per core
NEG = 0.01
SCALE = 1.0 / 64.0  # 1/sqrt(N)
NCORES = 8

_DT = mybir.dt.float32
MM_BF16 = True      # bf16 matmul inputs: full PE stream rate, half the DMA
SKIP_LDW = True     # s_i reuses the kp stationary loaded by s_r
_CACHE: dict = {}


def _build_nc():
    nc = bacc.Bacc("TRN2", target_bir_lowering=False, num_devices=NCORES)
    dt = _DT
    mdt = mybir.dt.bfloat16 if MM_BF16 else _DT  # matmul input dtype
    qrT = nc.dram_tensor("qrT", [P, N], mdt, kind="ExternalInput")
    qiT = nc.dram_tensor("qiT", [P, N], mdt, kind="ExternalInput")
    kp = nc.dram_tensor("kp", [P, NJPAR * JB], mdt, kind="ExternalInput")
    # va = 0.99 * V' (relu term), vb = 0.01 * V' (raw term, diagonal only)
    var_ = nc.dram_tensor("var", [P, NJPAR * F], mdt, kind="ExternalInput")
    vai = nc.dram_tensor("vai", [P, NJPAR * F], mdt, kind="ExternalInput")
    vbr = nc.dram_tensor("vbr", [P, NJPAR * F], mdt, kind="ExternalInput")
    vbi = nc.dram_tensor("vbi", [P, NJPAR * F], mdt, kind="ExternalInput")
    # per-slot correction weights: 0.01 * sum_{full J} kp_J @ V'_J  [P, 64]
    mcr = nc.dram_tensor("mcr", [P, NSLOT * F], mdt, kind="ExternalInput")
    mci = nc.dram_tensor("mci", [P, NSLOT * F], mdt, kind="ExternalInput")
    dmask = nc.dram_tensor("dmask", [2, JB, IBW], mdt, kind="ExternalInput")
    out = nc.dram_tensor("out", [P, N], mdt, kind="ExternalOutput")

    relu = mybir.ActivationFunctionType.Relu
    mul_op = mybir.AluOpType.mult

    with tile.TileContext(nc) as tc:
        with (
            tc.tile_pool(name="res", bufs=1) as res,
            tc.tile_pool(name="wp", bufs=10) as wp,
            tc.tile_pool(name="osb", bufs=2) as osb,
            tc.tile_pool(name="spsum", bufs=6, space="PSUM") as spsum,
            tc.tile_pool(name="ypsum", bufs=2, space="PSUM") as ypsum,
        ):
            sb_qr = res.tile([P, N], mdt, tag="qr")
            sb_qi = res.tile([P, N], mdt, tag="qi")
            sb_k = res.tile([P, NJPAR * JB], mdt, tag="k")
            sb_var = res.tile([P, NJPAR * F], mdt, tag="var")
            sb_vai = res.tile([P, NJPAR * F], mdt, tag="vai")
            sb_vbr = res.tile([P, NJPAR * F], mdt, tag="vbr")
            sb_vbi = res.tile([P, NJPAR * F], mdt, tag="vbi")
            sb_mcr = res.tile([P, NSLOT * F], mdt, tag="mcr")
            sb_mci = res.tile([P, NSLOT * F], mdt, tag="mci")
            sb_m0 = res.tile([JB, IBW], mdt, tag="m0")
            sb_m1 = res.tile([JB, IBW], mdt, tag="m1")

            def dma(dst, src, c):
                sl = slice(c * 512, (c + 1) * 512)
                nc.sync.dma_start(out=dst[:, sl], in_=src[:, sl])

            # slot-consumption order: slot s needs q chunk s, kp chunk
            # (2s+2)/4, v chunks from slot 4, mcr/mci from slot 1.
            nc.sync.dma_start(out=sb_m0, in_=dmask[0])
            nc.sync.dma_start(out=sb_m1, in_=dmask[1])
            dma(sb_k, kp, 0)
            dma(sb_qr, qrT, 0)
            dma(sb_qi, qiT, 0)
            for t_sb, t_dr in ((sb_var, var_), (sb_vai, vai),
                               (sb_vbr, vbr), (sb_vbi, vbi)):
                dma(t_sb, t_dr, 0)
            nc.sync.dma_start(out=sb_mcr, in_=mcr[:])
            nc.sync.dma_start(out=sb_mci, in_=mci[:])
            dma(sb_qr, qrT, 1)
            dma(sb_qi, qiT, 1)
            dma(sb_k, kp, 1)
            dma(sb_qr, qrT, 2)
            dma(sb_qi, qiT, 2)
            dma(sb_qr, qrT, 3)
            dma(sb_qi, qiT, 3)
            dma(sb_k, kp, 2)
            for t_sb, t_dr in ((sb_var, var_), (sb_vai, vai),
                               (sb_vbr, vbr), (sb_vbi, vbi)):
                dma(t_sb, t_dr, 1)
            dma(sb_qr, qrT, 4)
            dma(sb_qi, qiT, 4)
            dma(sb_k, kp, 3)
            for c in range(5, 8):
                dma(sb_qr, qrT, c)
                dma(sb_qi, qiT, c)

            sb_masks = (sb_m0, sb_m1)
            seq = [(s, p) for s in range(NSLOT) for p in range(2 * s + 2)]
            pend = {}    # idx -> per-comp drained tiles
            ytile = {}   # slot -> packed PSUM bank [P, IBW]
            drain_ctr = 0

            for idx in range(len(seq) + 2):
                if idx < len(seq):
                    s, p = seq[idx]
                    cnt = 2 * s + 2
                    isl = slice(s * IBW, (s + 1) * IBW)
                    if p == 0:
                        y = ytile[s] = ypsum.tile([P, IBW], dt, tag="y",
                                                  name=f"y{s}")
                        if s > 0:
                            msl = slice(s * F, (s + 1) * F)
                            nc.tensor.matmul(y[0:64, :], sb_mcr[:, msl],
                                             sb_qr[:, isl],
                                             start=True, stop=False)
                            nc.tensor.matmul(y[64:128, :], sb_mci[:, msl],
                                             sb_qi[:, isl],
                                             start=True, stop=False)
                    # scores: s_i reuses the kp stationary loaded by s_r
                    ksl = slice(p * JB, (p + 1) * JB)
                    s_r = spsum.tile([JB, IBW], dt, tag="s")
                    nc.tensor.matmul(s_r[:], sb_k[:, ksl], sb_qr[:, isl],
                                     start=True, stop=True)
                    s_i = spsum.tile([JB, IBW], dt, tag="s")
                    mm_i = nc.tensor.matmul(s_i[:], sb_k[:, ksl],
                                            sb_qi[:, isl],
                                            start=True, stop=True)
                    if SKIP_LDW:
                        mm_i.ins.ldweights = False
                    # drains (off the PE critical path; values lag 2 blocks)
                    tiles = []
                    for s_ps in (s_r, s_i):
                        if p < cnt - 2:
                            w = wp.tile([JB, IBW], mdt, tag="w")
                            if drain_ctr % 3 == 2:
                                nc.vector.tensor_scalar_max(w[:], s_ps[:], 0.0)
                            else:
                                nc.scalar.activation(w[:], s_ps[:], relu)
                            drain_ctr += 1
                            tiles.append((w,))
                        else:
                            mk = sb_masks[p - (cnt - 2)]
                            u = wp.tile([JB, IBW], mdt, tag="u")
                            nc.vector.tensor_tensor(out=u[:], in0=s_ps[:],
                                                    in1=mk[:], op=mul_op)
                            w = wp.tile([JB, IBW], mdt, tag="w")
                            nc.vector.tensor_scalar_max(w[:], u[:], 0.0)
                            tiles.append((u, w))
                    pend[idx] = tiles
                if idx >= 2:
                    s2, p2 = seq[idx - 2]
                    cnt2 = 2 * s2 + 2
                    y = ytile[s2]
                    vsl = slice(p2 * F, (p2 + 1) * F)
                    tiles = pend.pop(idx - 2)
                    for comp, (sb_va, sb_vb, psl) in enumerate((
                            (sb_var, sb_vbr, slice(0, 64)),
                            (sb_vai, sb_vbi, slice(64, 128)))):
                        first = (s2 == 0 and p2 == 0)
                        last = (p2 == cnt2 - 1)
                        tl = tiles[comp]
                        if len(tl) == 1:
                            nc.tensor.matmul(y[psl, :], sb_va[:, vsl], tl[0][:],
                                             start=first, stop=False)
                        else:
                            u, w = tl
                            nc.tensor.matmul(y[psl, :], sb_vb[:, vsl], u[:],
                                             start=first, stop=False)
                            nc.tensor.matmul(y[psl, :], sb_va[:, vsl], w[:],
                                             start=False, stop=last)
                    if p2 == cnt2 - 1:
                        isl2 = slice(s2 * IBW, (s2 + 1) * IBW)
                        y_sb = osb.tile([P, IBW], mdt, tag="ysb")
                        nc.scalar.copy(y_sb[:], y[:])
                        nc.sync.dma_start(out=out[:, isl2], in_=y_sb[:])
    nc.compile()
    return nc


def _prep_inputs(Q, K, V, W_att, b_att):
    """Host-side re-layout: per-core in_maps for run_bass_kernel_spmd."""
    Q = np.asarray(Q, dtype=np.float32)
    K = np.asarray(K, dtype=np.float32)
    V = np.asarray(V, dtype=np.float32)
    W_att = np.asarray(W_att, dtype=np.float32)

    Qf = Q.reshape(B, N, P)          # [b, i, f*2+c]
    Kf = K.reshape(B, N, P)
    Vpr = SCALE * (V[..., 0] @ W_att.T)   # [B, N, F]
    Vpi = SCALE * (V[..., 1] @ W_att.T)

    # causal masks for a slot's last two parity j-blocks, per core parity h:
    # diagonal sub-block d = 2k+h of the slot's group of 4
    jj = np.arange(JB)[:, None]
    ii = np.arange(IBW)[None, :]
    masks = {h: np.stack([(ii >= jj + JB * (2 * k + h)).astype(np.float32)
                          for k in range(2)]) for h in (0, 1)}

    if MM_BF16:
        import ml_dtypes
        cvt = lambda a: np.ascontiguousarray(a).astype(ml_dtypes.bfloat16)
    else:
        cvt = lambda a: np.ascontiguousarray(a, dtype=np.float32)

    in_maps = []
    for c in range(NCORES):
        b, h = divmod(c, 2)
        Qmodr = Qf[b].copy()
        Qmodr[:, 1::2] *= -1.0
        Qmodi = np.empty_like(Qf[b])
        Qmodi[:, 0::2] = Qf[b][:, 1::2]
        Qmodi[:, 1::2] = Qf[b][:, 0::2]
        # parity-packed K: [P, NJPAR*JB], position pp holds block J = 2*pp+h
        kp3 = Kf[b].reshape(N // JB, JB, P)[h::2]          # [16, j, p]
        kp = kp3.transpose(2, 0, 1).reshape(P, -1)         # [p, pp*JB+j]
        vr3 = Vpr[b].reshape(N // JB, JB, F)[h::2]         # [16, j, f]
        vi3 = Vpi[b].reshape(N // JB, JB, F)[h::2]
        vpr = vr3.transpose(1, 0, 2).reshape(JB, -1)       # [j, pp*F+f]
        vpi = vi3.transpose(1, 0, 2).reshape(JB, -1)
        # per-slot correction: 0.01 * sum over FULL blocks (pos < cnt-2 = 2s)
        prod_r = np.einsum('bjp,bjf->bpf', kp3, vr3)       # [16, p, f]
        prod_i = np.einsum('bjp,bjf->bpf', kp3, vi3)
        pre_r = np.concatenate(
            [np.zeros((1, P, F), np.float32), np.cumsum(prod_r, axis=0)])
        pre_i = np.concatenate(
            [np.zeros((1, P, F), np.float32), np.cumsum(prod_i, axis=0)])
        mcr = np.concatenate([NEG * pre_r[2 * s] for s in range(NSLOT)], axis=1)
        mci = np.concatenate([NEG * pre_i[2 * s] for s in range(NSLOT)], axis=1)
        in_maps.append({
            "qrT": cvt(Qmodr.T),
            "qiT": cvt(Qmodi.T),
            "kp": cvt(kp),
            "var": cvt((1.0 - NEG) * vpr),
            "vai": cvt((1.0 - NEG) * vpi),
            "vbr": cvt(NEG * vpr),
            "vbi": cvt(NEG * vpi),
            "mcr": cvt(mcr),
            "mci": cvt(mci),
            "dmask": cvt(masks[h]),
        })
    return in_maps


def _gather(results, b_att):
    b_att = np.asarray(b_att, dtype=np.float32)
    out = np.empty((B, N, F, 2), dtype=np.float32)
    for b in range(B):
        y = (results[2 * b]["out"].astype(np.float32)
             + results[2 * b + 1]["out"].astype(np.float32))  # [128, N]
        out[b, :, :, 0] = y[0:64].T + b_att[None, :]
        out[b, :, :, 1] = y[64:128].T + b_att[None, :]
    return out


def kernel(Q, K, V, W_att, b_att):
    if "nc" not in _CACHE:
        _CACHE["nc"] = _build_nc()
    nc = _CACHE["nc"]
    in_maps = _prep_inputs(Q, K, V, W_att, b_att)
    res = run_bass_kernel_spmd(nc, in_maps, core_ids=list(range(NCORES)))
    return _gather(res.results, b_att)


# revision 5
# speedup vs baseline: 3.3910x; 1.1060x over previous
"""Trainium2 Bass kernel for nn_AttentionOutput (complex causal leaky-relu attention).

Reference (B=4, N=4096, F=64), per batch:
    sr = (Qr@Kr^T - Qi@Ki^T)/sqrt(N); si = (Qr@Ki^T + Qi@Kr^T)/sqrt(N)
    wr = tril * leaky_relu(sr);        wi = tril * leaky_relu(si)
    out_r = (wr@Vr)@W_att^T + b;       out_i = (wi@Vi)@W_att^T + b

Distribution: 2 cores per batch.  Core parity h processes j-blocks J === h
(mod 2) for ALL 4096 query rows; causal work is then identical across cores
(slot I needs 2I+2 j-blocks), so a single SPMD program serves all 8 cores and
the host sums the two partial outputs per batch.

Host-side layout prep removes every on-device transpose:
  - scores contract over p = f*2+c (128 partitions, ONE matmul per component):
    sr = Qmodr . K^T where Qmodr = Q with odd columns negated, and
    si = Qmodi . K^T where Qmodi = Q with column pairs swapped; K stays plain.
    Both Q variants are fed pre-transposed [128, N].
  - V' = (1/64) V @ W_att^T folds the score scale and the output projection
    into the attention-value matmul (leaky_relu is positively homogeneous).
  - output is stored transposed ([128, N]: y_r^T on rows 0:64, y_i^T on
    64:128); the host untransposes, interleaves, adds bias, sums parities.

leaky_relu lowering (RELU_CORR): leaky(s) = 0.99*relu(s) + 0.01*s.  For
causally-full j-blocks the 0.01*s term telescopes into a per-slot constant
matmul accumulated into the y PSUM bank (weights mcr/mci precomputed on the
host).  Diagonal tiles compute u = mask*s (drain), w = relu(u), and feed two
matmuls against 0.99*V' and 0.01*V'.

v2 pipeline (PE-throughput oriented):
  - flat (slot, j-block) sequence; score matmuls run TWO blocks ahead of the
    value matmuls so PSUM drains are never on the PE critical path.
  - s_r/s_i share the kp stationary: the s_i matmul is emitted with
    ldweights=False so the PE skips the redundant LDWEIGHTS.
  - y_r/y_i packed into ONE PSUM bank (partitions 0:64 / 64:128) via the PE
    column-tiling (out.base_partition=64 for the i half): halves y banks and
    output copies/DMAs.  6 score banks + 2 y banks = all 8 PSUM banks.
  - drains spread over three engines: full tiles 2:1 ACT:DVE, diagonal
    mask-mults on DVE, diagonal relus on GpSimd (SBUF->SBUF).
  - input DMA issued in slot-consumption order; output stored bf16.

NOTE: ACT Lrelu reading PSUM hangs TRN2 (empirically) — never emit it.
"""

import numpy as np

import concourse.bacc as bacc
import concourse.tile as tile
from concourse import mybir
from concourse.bass_utils import run_bass_kernel_spmd

B, N, F = 4, 4096, 64
P = 128             # = 2*F: score contraction width / partition count
JB = 128            # j-block width
IBW = 512           # i-block (slot) width
NSLOT = N // IBW    # 8 slots
NJPAR = N // JB // 2  # 16 parity j-blocks per core
NEG = 0.01
SCALE = 1.0 / 64.0  # 1/sqrt(N)
NCORES = 8

_DT = mybir.dt.float32
MM_BF16 = True      # bf16 matmul inputs: full PE stream rate, half the DMA
SKIP_LDW = True     # s_i reuses the kp stationary loaded by s_r
_CACHE: dict = {}


def _build_nc():
    nc = bacc.Bacc("TRN2", target_bir_lowering=False, num_devices=NCORES)
    dt = _DT
    mdt = mybir.dt.bfloat16 if MM_BF16 else _DT  # matmul input dtype
    qrT = nc.dram_tensor("qrT", [P, N], mdt, kind="ExternalInput")
    qiT = nc.dram_tensor("qiT", [P, N], mdt, kind="ExternalInput")
    kp = nc.dram_tensor("kp", [P, NJPAR * JB], mdt, kind="ExternalInput")
    # va = 0.99 * V' (relu term), vb = 0.01 * V' (raw term, diagonal only)
    var_ = nc.dram_tensor("var", [P, NJPAR * F], mdt, kind="ExternalInput")
    vai = nc.dram_tensor("vai", [P, NJPAR * F], mdt, kind="ExternalInput")
    # per-slot correction weights: 0.01 * sum_{full J} kp_J @ V'_J  [P, 64]
    mcr = nc.dram_tensor("mcr", [P, NSLOT * F], mdt, kind="ExternalInput")
    mci = nc.dram_tensor("mci", [P, NSLOT * F], mdt, kind="ExternalInput")
    dmask = nc.dram_tensor("dmask", [2, JB, IBW], mdt, kind="ExternalInput")
    out = nc.dram_tensor("out", [P, N], mdt, kind="ExternalOutput")

    relu = mybir.ActivationFunctionType.Relu
    lrelu = mybir.ActivationFunctionType.Lrelu
    mul_op = mybir.AluOpType.mult

    with tile.TileContext(nc) as tc:
        with (
            tc.tile_pool(name="res", bufs=1) as res,
            tc.tile_pool(name="wp", bufs=10) as wp,
            tc.tile_pool(name="osb", bufs=2) as osb,
            tc.tile_pool(name="spsum", bufs=6, space="PSUM") as spsum,
            tc.tile_pool(name="ypsum", bufs=2, space="PSUM") as ypsum,
        ):
            sb_qr = res.tile([P, N], mdt, tag="qr")
            sb_qi = res.tile([P, N], mdt, tag="qi")
            sb_k = res.tile([P, NJPAR * JB], mdt, tag="k")
            sb_var = res.tile([P, NJPAR * F], mdt, tag="var")
            sb_vai = res.tile([P, NJPAR * F], mdt, tag="vai")
            sb_mcr = res.tile([P, NSLOT * F], mdt, tag="mcr")
            sb_mci = res.tile([P, NSLOT * F], mdt, tag="mci")
            sb_m0 = res.tile([JB, IBW], mdt, tag="m0")
            sb_m1 = res.tile([JB, IBW], mdt, tag="m1")

            def dma(dst, src, c):
                sl = slice(c * 512, (c + 1) * 512)
                nc.sync.dma_start(out=dst[:, sl], in_=src[:, sl])

            # slot-consumption order: slot s needs q chunk s, kp chunk
            # (2s+2)/4, v chunks from slot 4, mcr/mci from slot 1.
            nc.sync.dma_start(out=sb_k[:, 0:JB], in_=kp[:, 0:JB])
            dma(sb_qr, qrT, 0)
            dma(sb_qi, qiT, 0)
            nc.sync.dma_start(out=sb_m0, in_=dmask[0])
            nc.sync.dma_start(out=sb_m1, in_=dmask[1])
            nc.sync.dma_start(out=sb_k[:, JB:512], in_=kp[:, JB:512])
            dma(sb_var, var_, 0)
            dma(sb_vai, vai, 0)
            nc.sync.dma_start(out=sb_mcr, in_=mcr[:])
            nc.sync.dma_start(out=sb_mci, in_=mci[:])
            dma(sb_qr, qrT, 1)
            dma(sb_qi, qiT, 1)
            dma(sb_k, kp, 1)
            dma(sb_qr, qrT, 2)
            dma(sb_qi, qiT, 2)
            dma(sb_qr, qrT, 3)
            dma(sb_qi, qiT, 3)
            dma(sb_k, kp, 2)
            dma(sb_var, var_, 1)
            dma(sb_vai, vai, 1)
            dma(sb_qr, qrT, 4)
            dma(sb_qi, qiT, 4)
            dma(sb_k, kp, 3)
            for c in range(5, 8):
                dma(sb_qr, qrT, c)
                dma(sb_qi, qiT, c)

            sb_masks = (sb_m0, sb_m1)
            seq = [(s, p) for s in range(NSLOT) for p in range(2 * s + 2)]
            pend = {}    # idx -> per-comp drained tiles
            ytile = {}   # slot -> packed PSUM bank [P, IBW]
            drain_ctr = 0

            for idx in range(len(seq) + 2):
                if idx < len(seq):
                    s, p = seq[idx]
                    cnt = 2 * s + 2
                    isl = slice(s * IBW, (s + 1) * IBW)
                    if p == 0:
                        y = ytile[s] = ypsum.tile([P, IBW], dt, tag="y",
                                                  name=f"y{s}")
                        if s > 0:
                            msl = slice(s * F, (s + 1) * F)
                            nc.tensor.matmul(y[0:64, :], sb_mcr[:, msl],
                                             sb_qr[:, isl],
                                             start=True, stop=False)
                            nc.tensor.matmul(y[64:128, :], sb_mci[:, msl],
                                             sb_qi[:, isl],
                                             start=True, stop=False)
                    # scores: s_i reuses the kp stationary loaded by s_r
                    ksl = slice(p * JB, (p + 1) * JB)
                    s_r = spsum.tile([JB, IBW], dt, tag="s")
                    nc.tensor.matmul(s_r[:], sb_k[:, ksl], sb_qr[:, isl],
                                     start=True, stop=True)
                    s_i = spsum.tile([JB, IBW], dt, tag="s")
                    mm_i = nc.tensor.matmul(s_i[:], sb_k[:, ksl],
                                            sb_qi[:, isl],
                                            start=True, stop=True)
                    if SKIP_LDW:
                        mm_i.ins.ldweights = False
                    # drains (off the PE critical path; values lag 2 blocks)
                    tiles = []
                    for s_ps in (s_r, s_i):
                        if p < cnt - 2:
                            w = wp.tile([JB, IBW], mdt, tag="w")
                            if drain_ctr % 2 == 1:
                                nc.vector.tensor_scalar_max(w[:], s_ps[:], 0.0)
                            else:
                                nc.scalar.activation(w[:], s_ps[:], relu)
                            drain_ctr += 1
                            tiles.append(w)
                        else:
                            # mask pre-scaled by 1/0.99; Lrelu(u)*0.99V' ==
                            # V'*leaky(mask*s) by positive homogeneity
                            mk = sb_masks[p - (cnt - 2)]
                            u = wp.tile([JB, IBW], mdt, tag="u")
                            nc.vector.tensor_tensor(out=u[:], in0=s_ps[:],
                                                    in1=mk[:], op=mul_op)
                            w = wp.tile([JB, IBW], mdt, tag="w")
                            nc.scalar.activation(w[:], u[:], lrelu)
                            tiles.append(w)
                    pend[idx] = tiles
                if idx >= 2:
                    s2, p2 = seq[idx - 2]
                    cnt2 = 2 * s2 + 2
                    y = ytile[s2]
                    vsl = slice(p2 * F, (p2 + 1) * F)
                    tiles = pend.pop(idx - 2)
                    for comp, (sb_va, psl) in enumerate((
                            (sb_var, slice(0, 64)),
                            (sb_vai, slice(64, 128)))):
                        first = (s2 == 0 and p2 == 0)
                        last = (p2 == cnt2 - 1)
                        nc.tensor.matmul(y[psl, :], sb_va[:, vsl],
                                         tiles[comp][:],
                                         start=first, stop=last)
                    if p2 == cnt2 - 1:
                        isl2 = slice(s2 * IBW, (s2 + 1) * IBW)
                        y_sb = osb.tile([P, IBW], mdt, tag="ysb")
                        nc.scalar.copy(y_sb[:], y[:])
                        nc.sync.dma_start(out=out[:, isl2], in_=y_sb[:])
    nc.compile()
    return nc


def _prep_inputs(Q, K, V, W_att, b_att):
    """Host-side re-layout: per-core in_maps for run_bass_kernel_spmd."""
    Q = np.asarray(Q, dtype=np.float32)
    K = np.asarray(K, dtype=np.float32)
    V = np.asarray(V, dtype=np.float32)
    W_att = np.asarray(W_att, dtype=np.float32)

    Qf = Q.reshape(B, N, P)          # [b, i, f*2+c]
    Kf = K.reshape(B, N, P)
    Vpr = SCALE * (V[..., 0] @ W_att.T)   # [B, N, F]
    Vpi = SCALE * (V[..., 1] @ W_att.T)

    # causal masks for a slot's last two parity j-blocks, per core parity h:
    # diagonal sub-block d = 2k+h of the slot's group of 4
    jj = np.arange(JB)[:, None]
    ii = np.arange(IBW)[None, :]
    mscale = 1.0 / (1.0 - NEG)
    masks = {h: np.stack([mscale * (ii >= jj + JB * (2 * k + h))
                          .astype(np.float32) for k in range(2)])
             for h in (0, 1)}

    if MM_BF16:
        import ml_dtypes
        cvt = lambda a: np.ascontiguousarray(a).astype(ml_dtypes.bfloat16)
    else:
        cvt = lambda a: np.ascontiguousarray(a, dtype=np.float32)

    in_maps = []
    for c in range(NCORES):
        b, h = divmod(c, 2)
        Qmodr = Qf[b].copy()
        Qmodr[:, 1::2] *= -1.0
        Qmodi = np.empty_like(Qf[b])
        Qmodi[:, 0::2] = Qf[b][:, 1::2]
        Qmodi[:, 1::2] = Qf[b][:, 0::2]
        # parity-packed K: [P, NJPAR*JB], position pp holds block J = 2*pp+h
        kp3 = Kf[b].reshape(N // JB, JB, P)[h::2]          # [16, j, p]
        kp = kp3.transpose(2, 0, 1).reshape(P, -1)         # [p, pp*JB+j]
        vr3 = Vpr[b].reshape(N // JB, JB, F)[h::2]         # [16, j, f]
        vi3 = Vpi[b].reshape(N // JB, JB, F)[h::2]
        vpr = vr3.transpose(1, 0, 2).reshape(JB, -1)       # [j, pp*F+f]
        vpi = vi3.transpose(1, 0, 2).reshape(JB, -1)
        # per-slot correction: 0.01 * sum over FULL blocks (pos < cnt-2 = 2s)
        prod_r = np.einsum('bjp,bjf->bpf', kp3, vr3)       # [16, p, f]
        prod_i = np.einsum('bjp,bjf->bpf', kp3, vi3)
        pre_r = np.concatenate(
            [np.zeros((1, P, F), np.float32), np.cumsum(prod_r, axis=0)])
        pre_i = np.concatenate(
            [np.zeros((1, P, F), np.float32), np.cumsum(prod_i, axis=0)])
        mcr = np.concatenate([NEG * pre_r[2 * s] for s in range(NSLOT)], axis=1)
        mci = np.concatenate([NEG * pre_i[2 * s] for s in range(NSLOT)], axis=1)
        in_maps.append({
            "qrT": cvt(Qmodr.T),
            "qiT": cvt(Qmodi.T),
            "kp": cvt(kp),
            "var": cvt((1.0 - NEG) * vpr),
            "vai": cvt((1.0 - NEG) * vpi),
            "mcr": cvt(mcr),
            "mci": cvt(mci),
            "dmask": cvt(masks[h]),
        })
    return in_maps


def _gather(results, b_att):
    b_att = np.asarray(b_att, dtype=np.float32)
    out = np.empty((B, N, F, 2), dtype=np.float32)
    for b in range(B):
        y = (results[2 * b]["out"].astype(np.float32)
             + results[2 * b + 1]["out"].astype(np.float32))  # [128, N]
        out[b, :, :, 0] = y[0:64].T + b_att[None, :]
        out[b, :, :, 1] = y[64:128].T + b_att[None, :]
    return out


def kernel(Q, K, V, W_att, b_att):
    if "nc" not in _CACHE:
        _CACHE["nc"] = _build_nc()
    nc = _CACHE["nc"]
    in_maps = _prep_inputs(Q, K, V, W_att, b_att)
    res = run_bass_kernel_spmd(nc, in_maps, core_ids=list(range(NCORES)))
    return _gather(res.results, b_att)


# revision 6
# speedup vs baseline: 3.4767x; 1.0253x over previous
"""Trainium2 Bass kernel for nn_AttentionOutput (complex causal leaky-relu attention).

Reference (B=4, N=4096, F=64), per batch:
    sr = (Qr@Kr^T - Qi@Ki^T)/sqrt(N); si = (Qr@Ki^T + Qi@Kr^T)/sqrt(N)
    wr = tril * leaky_relu(sr);        wi = tril * leaky_relu(si)
    out_r = (wr@Vr)@W_att^T + b;       out_i = (wi@Vi)@W_att^T + b

Distribution: 2 cores per batch.  Core parity h processes j-blocks J === h
(mod 2) for ALL 4096 query rows; causal work is then identical across cores
(slot I needs 2I+2 j-blocks), so a single SPMD program serves all 8 cores and
the host sums the two partial outputs per batch.

Host-side layout prep removes every on-device transpose:
  - scores contract over p = f*2+c (128 partitions, ONE matmul per component):
    sr = Qmodr . K^T where Qmodr = Q with odd columns negated, and
    si = Qmodi . K^T where Qmodi = Q with column pairs swapped; K stays plain.
    Both Q variants are fed pre-transposed [128, N].
  - V' = (1/64) V @ W_att^T folds the score scale and the output projection
    into the attention-value matmul (leaky_relu is positively homogeneous).
  - output is stored transposed ([128, N]: y_r^T on rows 0:64, y_i^T on
    64:128); the host untransposes, interleaves, adds bias, sums parities.

leaky_relu lowering (RELU_CORR): leaky(s) = 0.99*relu(s) + 0.01*s.  For
causally-full j-blocks the 0.01*s term telescopes into a per-slot constant
matmul accumulated into the y PSUM bank (weights mcr/mci precomputed on the
host).  Diagonal tiles compute u = mask*s (drain), w = relu(u), and feed two
matmuls against 0.99*V' and 0.01*V'.

v2 pipeline (PE-throughput oriented):
  - flat (slot, j-block) sequence; score matmuls run TWO blocks ahead of the
    value matmuls so PSUM drains are never on the PE critical path.
  - s_r/s_i share the kp stationary: the s_i matmul is emitted with
    ldweights=False so the PE skips the redundant LDWEIGHTS.
  - y_r/y_i packed into ONE PSUM bank (partitions 0:64 / 64:128) via the PE
    column-tiling (out.base_partition=64 for the i half): halves y banks and
    output copies/DMAs.  6 score banks + 2 y banks = all 8 PSUM banks.
  - drains spread over three engines: full tiles 2:1 ACT:DVE, diagonal
    mask-mults on DVE, diagonal relus on GpSimd (SBUF->SBUF).
  - input DMA issued in slot-consumption order; output stored bf16.

NOTE: ACT Lrelu reading PSUM hangs TRN2 (empirically) — never emit it.
"""

import numpy as np

import concourse.bacc as bacc
import concourse.tile as tile
from concourse import mybir
from concourse.bass_utils import run_bass_kernel_spmd

B, N, F = 4, 4096, 64
P = 128             # = 2*F: score contraction width / partition count
JB = 128            # j-block width
IBW = 512           # i-block (slot) width
NSLOT = N // IBW    # 8 slots
NJPAR = N // JB // 2  # 16 parity j-blocks per core
NEG = 0.01
SCALE = 1.0 / 64.0  # 1/sqrt(N)
NCORES = 8

_DT = mybir.dt.float32
MM_BF16 = True      # bf16 matmul inputs: full PE stream rate, half the DMA
SKIP_LDW = True     # s_i reuses the kp stationary loaded by s_r
_CACHE: dict = {}


def _build_nc():
    nc = bacc.Bacc("TRN2", target_bir_lowering=False, num_devices=NCORES)
    dt = _DT
    mdt = mybir.dt.bfloat16 if MM_BF16 else _DT  # matmul input dtype
    qrT = nc.dram_tensor("qrT", [P, N], mdt, kind="ExternalInput")
    qiT = nc.dram_tensor("qiT", [P, N], mdt, kind="ExternalInput")
    kp = nc.dram_tensor("kp", [P, NJPAR * JB], mdt, kind="ExternalInput")
    # va = 0.99 * V' (relu term), vb = 0.01 * V' (raw term, diagonal only)
    var_ = nc.dram_tensor("var", [P, NJPAR * F], mdt, kind="ExternalInput")
    vai = nc.dram_tensor("vai", [P, NJPAR * F], mdt, kind="ExternalInput")
    # per-slot correction weights: 0.01 * sum_{full J} kp_J @ V'_J  [P, 64]
    mcr = nc.dram_tensor("mcr", [P, NSLOT * F], mdt, kind="ExternalInput")
    mci = nc.dram_tensor("mci", [P, NSLOT * F], mdt, kind="ExternalInput")
    dmask = nc.dram_tensor("dmask", [2, JB, IBW], mdt, kind="ExternalInput")
    out = nc.dram_tensor("out", [P, N], mdt, kind="ExternalOutput")

    relu = mybir.ActivationFunctionType.Relu
    lrelu = mybir.ActivationFunctionType.Lrelu
    mul_op = mybir.AluOpType.mult

    with tile.TileContext(nc) as tc:
        with (
            tc.tile_pool(name="res", bufs=1) as res,
            tc.tile_pool(name="wp", bufs=10) as wp,
            tc.tile_pool(name="osb", bufs=2) as osb,
            tc.tile_pool(name="spsum", bufs=6, space="PSUM") as spsum,
            tc.tile_pool(name="ypsum", bufs=2, space="PSUM") as ypsum,
        ):
            sb_qr = res.tile([P, N], mdt, tag="qr")
            sb_qi = res.tile([P, N], mdt, tag="qi")
            sb_k = res.tile([P, NJPAR * JB], mdt, tag="k")
            sb_var = res.tile([P, NJPAR * F], mdt, tag="var")
            sb_vai = res.tile([P, NJPAR * F], mdt, tag="vai")
            sb_mcr = res.tile([P, NSLOT * F], mdt, tag="mcr")
            sb_mci = res.tile([P, NSLOT * F], mdt, tag="mci")
            sb_m0 = res.tile([JB, IBW], mdt, tag="m0")
            sb_m1 = res.tile([JB, IBW], mdt, tag="m1")

            def dma(dst, src, c):
                sl = slice(c * 512, (c + 1) * 512)
                nc.sync.dma_start(out=dst[:, sl], in_=src[:, sl])

            # DMAs ordered by first-use time, split across the two HW DGE
            # queues (Sync + Activation) so the head loads in parallel.
            # Sync queue: critical path of the first score matmuls.
            nc.sync.dma_start(out=sb_k[:, 0:JB], in_=kp[:, 0:JB])
            dma(sb_qr, qrT, 0)
            # Activation queue: first diag masks + slot-0 value weights +
            # corr weights (ACT is idle until its first drain ~12us in).
            nc.scalar.dma_start(out=sb_qi[:, 0:512], in_=qiT[:, 0:512])
            nc.scalar.dma_start(out=sb_m0, in_=dmask[0])
            nc.scalar.dma_start(out=sb_m1, in_=dmask[1])
            nc.scalar.dma_start(out=sb_var[:, 0:512], in_=var_[:, 0:512])
            nc.scalar.dma_start(out=sb_vai[:, 0:512], in_=vai[:, 0:512])
            nc.scalar.dma_start(out=sb_mcr, in_=mcr[:])
            nc.scalar.dma_start(out=sb_mci, in_=mci[:])
            # Sync queue: remainder in slot order.
            nc.sync.dma_start(out=sb_k[:, JB:512], in_=kp[:, JB:512])
            dma(sb_qr, qrT, 1)
            dma(sb_qi, qiT, 1)
            dma(sb_k, kp, 1)
            dma(sb_qr, qrT, 2)
            dma(sb_qi, qiT, 2)
            dma(sb_qr, qrT, 3)
            dma(sb_qi, qiT, 3)
            dma(sb_k, kp, 2)
            dma(sb_var, var_, 1)
            dma(sb_vai, vai, 1)
            dma(sb_qr, qrT, 4)
            dma(sb_qi, qiT, 4)
            dma(sb_k, kp, 3)
            for c in range(5, 8):
                dma(sb_qr, qrT, c)
                dma(sb_qi, qiT, c)

            sb_masks = (sb_m0, sb_m1)
            seq = [(s, p) for s in range(NSLOT) for p in range(2 * s + 2)]
            pend = {}    # idx -> per-comp drained tiles
            ytile = {}   # slot -> packed PSUM bank [P, IBW]
            drain_ctr = 0

            for idx in range(len(seq) + 2):
                if idx < len(seq):
                    s, p = seq[idx]
                    cnt = 2 * s + 2
                    isl = slice(s * IBW, (s + 1) * IBW)
                    if p == 0:
                        y = ytile[s] = ypsum.tile([P, IBW], dt, tag="y",
                                                  name=f"y{s}")
                        if s > 0:
                            msl = slice(s * F, (s + 1) * F)
                            nc.tensor.matmul(y[0:64, :], sb_mcr[:, msl],
                                             sb_qr[:, isl],
                                             start=True, stop=False)
                            nc.tensor.matmul(y[64:128, :], sb_mci[:, msl],
                                             sb_qi[:, isl],
                                             start=True, stop=False)
                    # scores: s_i reuses the kp stationary loaded by s_r
                    ksl = slice(p * JB, (p + 1) * JB)
                    s_r = spsum.tile([JB, IBW], dt, tag="s")
                    nc.tensor.matmul(s_r[:], sb_k[:, ksl], sb_qr[:, isl],
                                     start=True, stop=True)
                    s_i = spsum.tile([JB, IBW], dt, tag="s")
                    mm_i = nc.tensor.matmul(s_i[:], sb_k[:, ksl],
                                            sb_qi[:, isl],
                                            start=True, stop=True)
                    if SKIP_LDW:
                        mm_i.ins.ldweights = False
                    # drains (off the PE critical path; values lag 2 blocks)
                    tiles = []
                    for s_ps in (s_r, s_i):
                        if p < cnt - 2:
                            w = wp.tile([JB, IBW], mdt, tag="w")
                            if drain_ctr % 2 == 1:
                                nc.vector.tensor_scalar_max(w[:], s_ps[:], 0.0)
                            else:
                                nc.scalar.activation(w[:], s_ps[:], relu)
                            drain_ctr += 1
                            tiles.append(w)
                        else:
                            # mask pre-scaled by 1/0.99; Lrelu(u)*0.99V' ==
                            # V'*leaky(mask*s) by positive homogeneity
                            mk = sb_masks[p - (cnt - 2)]
                            u = wp.tile([JB, IBW], mdt, tag="u")
                            nc.vector.tensor_tensor(out=u[:], in0=s_ps[:],
                                                    in1=mk[:], op=mul_op)
                            w = wp.tile([JB, IBW], mdt, tag="w")
                            nc.scalar.activation(w[:], u[:], lrelu)
                            tiles.append(w)
                    pend[idx] = tiles
                if idx >= 2:
                    s2, p2 = seq[idx - 2]
                    cnt2 = 2 * s2 + 2
                    y = ytile[s2]
                    vsl = slice(p2 * F, (p2 + 1) * F)
                    tiles = pend.pop(idx - 2)
                    for comp, (sb_va, psl) in enumerate((
                            (sb_var, slice(0, 64)),
                            (sb_vai, slice(64, 128)))):
                        first = (s2 == 0 and p2 == 0)
                        last = (p2 == cnt2 - 1)
                        nc.tensor.matmul(y[psl, :], sb_va[:, vsl],
                                         tiles[comp][:],
                                         start=first, stop=last)
                    if p2 == cnt2 - 1:
                        isl2 = slice(s2 * IBW, (s2 + 1) * IBW)
                        y_sb = osb.tile([P, IBW], mdt, tag="ysb")
                        nc.scalar.copy(y_sb[:], y[:])
                        nc.sync.dma_start(out=out[:, isl2], in_=y_sb[:])
    nc.compile()
    return nc


def _prep_inputs(Q, K, V, W_att, b_att):
    """Host-side re-layout: per-core in_maps for run_bass_kernel_spmd."""
    Q = np.asarray(Q, dtype=np.float32)
    K = np.asarray(K, dtype=np.float32)
    V = np.asarray(V, dtype=np.float32)
    W_att = np.asarray(W_att, dtype=np.float32)

    Qf = Q.reshape(B, N, P)          # [b, i, f*2+c]
    Kf = K.reshape(B, N, P)
    Vpr = SCALE * (V[..., 0] @ W_att.T)   # [B, N, F]
    Vpi = SCALE * (V[..., 1] @ W_att.T)

    # causal masks for a slot's last two parity j-blocks, per core parity h:
    # diagonal sub-block d = 2k+h of the slot's group of 4
    jj = np.arange(JB)[:, None]
    ii = np.arange(IBW)[None, :]
    mscale = 1.0 / (1.0 - NEG)
    masks = {h: np.stack([mscale * (ii >= jj + JB * (2 * k + h))
                          .astype(np.float32) for k in range(2)])
             for h in (0, 1)}

    if MM_BF16:
        import ml_dtypes
        cvt = lambda a: np.ascontiguousarray(a).astype(ml_dtypes.bfloat16)
    else:
        cvt = lambda a: np.ascontiguousarray(a, dtype=np.float32)

    in_maps = []
    for c in range(NCORES):
        b, h = divmod(c, 2)
        Qmodr = Qf[b].copy()
        Qmodr[:, 1::2] *= -1.0
        Qmodi = np.empty_like(Qf[b])
        Qmodi[:, 0::2] = Qf[b][:, 1::2]
        Qmodi[:, 1::2] = Qf[b][:, 0::2]
        # parity-packed K: [P, NJPAR*JB], position pp holds block J = 2*pp+h
        kp3 = Kf[b].reshape(N // JB, JB, P)[h::2]          # [16, j, p]
        kp = kp3.transpose(2, 0, 1).reshape(P, -1)         # [p, pp*JB+j]
        vr3 = Vpr[b].reshape(N // JB, JB, F)[h::2]         # [16, j, f]
        vi3 = Vpi[b].reshape(N // JB, JB, F)[h::2]
        vpr = vr3.transpose(1, 0, 2).reshape(JB, -1)       # [j, pp*F+f]
        vpi = vi3.transpose(1, 0, 2).reshape(JB, -1)
        # per-slot correction: 0.01 * sum over FULL blocks (pos < cnt-2 = 2s)
        prod_r = np.einsum('bjp,bjf->bpf', kp3, vr3)       # [16, p, f]
        prod_i = np.einsum('bjp,bjf->bpf', kp3, vi3)
        pre_r = np.concatenate(
            [np.zeros((1, P, F), np.float32), np.cumsum(prod_r, axis=0)])
        pre_i = np.concatenate(
            [np.zeros((1, P, F), np.float32), np.cumsum(prod_i, axis=0)])
        mcr = np.concatenate([NEG * pre_r[2 * s] for s in range(NSLOT)], axis=1)
        mci = np.concatenate([NEG * pre_i[2 * s] for s in range(NSLOT)], axis=1)
        in_maps.append({
            "qrT": cvt(Qmodr.T),
            "qiT": cvt(Qmodi.T),
            "kp": cvt(kp),
            "var": cvt((1.0 - NEG) * vpr),
            "vai": cvt((1.0 - NEG) * vpi),
            "mcr": cvt(mcr),
            "mci": cvt(mci),
            "dmask": cvt(masks[h]),
        })
    return in_maps


def _gather(results, b_att):
    b_att = np.asarray(b_att, dtype=np.float32)
    out = np.empty((B, N, F, 2), dtype=np.float32)
    for b in range(B):
        y = (results[2 * b]["out"].astype(np.float32)
             + results[2 * b + 1]["out"].astype(np.float32))  # [128, N]
        out[b, :, :, 0] = y[0:64].T + b_att[None, :]
        out[b, :, :, 1] = y[64:128].T + b_att[None, :]
    return out


def kernel(Q, K, V, W_att, b_att):
    if "nc" not in _CACHE:
        _CACHE["nc"] = _build_nc()
    nc = _CACHE["nc"]
    in_maps = _prep_inputs(Q, K, V, W_att, b_att)
    res = run_bass_kernel_spmd(nc, in_maps, core_ids=list(range(NCORES)))
    return _gather(res.results, b_att)


# revision 7
# speedup vs baseline: 3.5178x; 1.0118x over previous
"""Trainium2 Bass kernel for nn_AttentionOutput (complex causal leaky-relu attention).

Reference (B=4, N=4096, F=64), per batch:
    sr = (Qr@Kr^T - Qi@Ki^T)/sqrt(N); si = (Qr@Ki^T + Qi@Kr^T)/sqrt(N)
    wr = tril * leaky_relu(sr);        wi = tril * leaky_relu(si)
    out_r = (wr@Vr)@W_att^T + b;       out_i = (wi@Vi)@W_att^T + b

Distribution: 2 cores per batch.  Core parity h processes j-blocks J === h
(mod 2) for ALL 4096 query rows; causal work is then identical across cores
(slot I needs 2I+2 j-blocks), so a single SPMD program serves all 8 cores and
the host sums the two partial outputs per batch.

Host-side layout prep removes every on-device transpose:
  - scores contract over p = f*2+c (128 partitions, ONE matmul per component):
    sr = Qmodr . K^T where Qmodr = Q with odd columns negated, and
    si = Qmodi . K^T where Qmodi = Q with column pairs swapped; K stays plain.
    Both Q variants are fed pre-transposed [128, N].
  - V' = (1/64) V @ W_att^T folds the score scale and the output projection
    into the attention-value matmul (leaky_relu is positively homogeneous).
  - output is stored transposed ([128, N]: y_r^T on rows 0:64, y_i^T on
    64:128); the host untransposes, interleaves, adds bias, sums parities.

leaky_relu lowering (RELU_CORR): leaky(s) = 0.99*relu(s) + 0.01*s.  For
causally-full j-blocks the 0.01*s term telescopes into a per-slot constant
matmul accumulated into the y PSUM bank (weights mcr/mci precomputed on the
host).  Diagonal tiles compute u = mask*s (drain), w = relu(u), and feed two
matmuls against 0.99*V' and 0.01*V'.

v2 pipeline (PE-throughput oriented):
  - flat (slot, j-block) sequence; score matmuls run TWO blocks ahead of the
    value matmuls so PSUM drains are never on the PE critical path.
  - s_r/s_i share the kp stationary: the s_i matmul is emitted with
    ldweights=False so the PE skips the redundant LDWEIGHTS.
  - y_r/y_i packed into ONE PSUM bank (partitions 0:64 / 64:128) via the PE
    column-tiling (out.base_partition=64 for the i half): halves y banks and
    output copies/DMAs.  6 score banks + 2 y banks = all 8 PSUM banks.
  - drains spread over three engines: full tiles 2:1 ACT:DVE, diagonal
    mask-mults on DVE, diagonal relus on GpSimd (SBUF->SBUF).
  - input DMA issued in slot-consumption order; output stored bf16.

NOTE: ACT Lrelu reading PSUM hangs TRN2 (empirically) — never emit it.
"""

import numpy as np

import concourse.bacc as bacc
import concourse.tile as tile
from concourse import mybir
from concourse.bass_utils import run_bass_kernel_spmd

B, N, F = 4, 4096, 64
P = 128             # = 2*F: score contraction width / partition count
JB = 128            # j-block width
IBW = 512           # i-block (slot) width
NSLOT = N // IBW    # 8 slots
NJPAR = N // JB // 2  # 16 parity j-blocks per core
NEG = 0.01
SCALE = 1.0 / 64.0  # 1/sqrt(N)
NCORES = 8

_DT = mybir.dt.float32
MM_BF16 = True      # bf16 matmul inputs: full PE stream rate, half the DMA
SKIP_LDW = True     # s_i reuses the kp stationary loaded by s_r
_CACHE: dict = {}


def _build_nc():
    nc = bacc.Bacc("TRN2", target_bir_lowering=False, num_devices=NCORES)
    dt = _DT
    mdt = mybir.dt.bfloat16 if MM_BF16 else _DT  # matmul input dtype
    qrT = nc.dram_tensor("qrT", [P, N], mdt, kind="ExternalInput")
    qiT = nc.dram_tensor("qiT", [P, N], mdt, kind="ExternalInput")
    kp = nc.dram_tensor("kp", [P, NJPAR * JB], mdt, kind="ExternalInput")
    # va = 0.99 * V' (relu term), vb = 0.01 * V' (raw term, diagonal only)
    var_ = nc.dram_tensor("var", [P, NJPAR * F], mdt, kind="ExternalInput")
    vai = nc.dram_tensor("vai", [P, NJPAR * F], mdt, kind="ExternalInput")
    # per-slot correction weights: 0.01 * sum_{full J} kp_J @ V'_J  [P, 64]
    mcr = nc.dram_tensor("mcr", [P, NSLOT * F], mdt, kind="ExternalInput")
    mci = nc.dram_tensor("mci", [P, NSLOT * F], mdt, kind="ExternalInput")
    dmask = nc.dram_tensor("dmask", [2, JB, IBW], mdt, kind="ExternalInput")
    out = nc.dram_tensor("out", [P, N], mdt, kind="ExternalOutput")

    relu = mybir.ActivationFunctionType.Relu
    lrelu = mybir.ActivationFunctionType.Lrelu
    mul_op = mybir.AluOpType.mult

    with tile.TileContext(nc) as tc:
        with (
            tc.tile_pool(name="res", bufs=1) as res,
            tc.tile_pool(name="wp", bufs=10) as wp,
            tc.tile_pool(name="osb", bufs=2) as osb,
            tc.tile_pool(name="spsum", bufs=6, space="PSUM") as spsum,
            tc.tile_pool(name="ypsum", bufs=2, space="PSUM") as ypsum,
        ):
            sb_qr = res.tile([P, N], mdt, tag="qr")
            sb_qi = res.tile([P, N], mdt, tag="qi")
            sb_k = res.tile([P, NJPAR * JB], mdt, tag="k")
            sb_var = res.tile([P, NJPAR * F], mdt, tag="var")
            sb_vai = res.tile([P, NJPAR * F], mdt, tag="vai")
            sb_mcr = res.tile([P, NSLOT * F], mdt, tag="mcr")
            sb_mci = res.tile([P, NSLOT * F], mdt, tag="mci")
            sb_m0 = res.tile([JB, IBW], mdt, tag="m0")
            sb_m1 = res.tile([JB, IBW], mdt, tag="m1")

            def dma(dst, src, c):
                sl = slice(c * 512, (c + 1) * 512)
                nc.sync.dma_start(out=dst[:, sl], in_=src[:, sl])

            # DMAs ordered by first-use time, split across the two HW DGE
            # queues (Sync + Activation) so the head loads in parallel.
            # Sync queue: critical path of the first score matmuls.
            dma(sb_qr, qrT, 0)
            nc.sync.dma_start(out=sb_k[:, 0:JB], in_=kp[:, 0:JB])
            # Activation queue: first diag masks + slot-0 value weights +
            # corr weights (ACT is idle until its first drain ~12us in).
            nc.scalar.dma_start(out=sb_qi[:, 0:512], in_=qiT[:, 0:512])
            nc.scalar.dma_start(out=sb_m0, in_=dmask[0])
            nc.scalar.dma_start(out=sb_m1, in_=dmask[1])
            nc.scalar.dma_start(out=sb_var[:, 0:512], in_=var_[:, 0:512])
            nc.scalar.dma_start(out=sb_vai[:, 0:512], in_=vai[:, 0:512])
            nc.scalar.dma_start(out=sb_mcr, in_=mcr[:])
            nc.scalar.dma_start(out=sb_mci, in_=mci[:])
            # Sync queue: remainder in slot order.
            nc.sync.dma_start(out=sb_k[:, JB:512], in_=kp[:, JB:512])
            dma(sb_qr, qrT, 1)
            dma(sb_qi, qiT, 1)
            dma(sb_k, kp, 1)
            dma(sb_qr, qrT, 2)
            dma(sb_qi, qiT, 2)
            dma(sb_qr, qrT, 3)
            dma(sb_qi, qiT, 3)
            dma(sb_k, kp, 2)
            dma(sb_var, var_, 1)
            dma(sb_vai, vai, 1)
            dma(sb_qr, qrT, 4)
            dma(sb_qi, qiT, 4)
            dma(sb_k, kp, 3)
            nc.sync.dma_start(out=sb_qr[:, 2560:4096], in_=qrT[:, 2560:4096])
            nc.sync.dma_start(out=sb_qi[:, 2560:4096], in_=qiT[:, 2560:4096])

            sb_masks = (sb_m0, sb_m1)
            seq = [(s, p) for s in range(NSLOT) for p in range(2 * s + 2)]
            pend = {}    # idx -> per-comp drained tiles
            ytile = {}   # slot -> packed PSUM bank [P, IBW]
            drain_ctr = 0

            for idx in range(len(seq) + 2):
                if idx < len(seq):
                    s, p = seq[idx]
                    cnt = 2 * s + 2
                    isl = slice(s * IBW, (s + 1) * IBW)
                    if p == 0:
                        y = ytile[s] = ypsum.tile([P, IBW], dt, tag="y",
                                                  name=f"y{s}")
                    # scores: s_i reuses the kp stationary loaded by s_r
                    ksl = slice(p * JB, (p + 1) * JB)
                    s_r = spsum.tile([JB, IBW], dt, tag="s")
                    nc.tensor.matmul(s_r[:], sb_k[:, ksl], sb_qr[:, isl],
                                     start=True, stop=True)
                    s_i = spsum.tile([JB, IBW], dt, tag="s")
                    mm_i = nc.tensor.matmul(s_i[:], sb_k[:, ksl],
                                            sb_qi[:, isl],
                                            start=True, stop=True)
                    if SKIP_LDW:
                        mm_i.ins.ldweights = False
                    # drains (off the PE critical path; values lag 2 blocks)
                    tiles = []
                    for s_ps in (s_r, s_i):
                        if p < cnt - 2:
                            w = wp.tile([JB, IBW], mdt, tag="w")
                            if drain_ctr % 2 == 1:
                                nc.vector.tensor_scalar_max(w[:], s_ps[:], 0.0)
                            else:
                                nc.scalar.activation(w[:], s_ps[:], relu)
                            drain_ctr += 1
                            tiles.append(w)
                        else:
                            # mask pre-scaled by 1/0.99; Lrelu(u)*0.99V' ==
                            # V'*leaky(mask*s) by positive homogeneity
                            mk = sb_masks[p - (cnt - 2)]
                            u = wp.tile([JB, IBW], mdt, tag="u")
                            nc.vector.tensor_tensor(out=u[:], in0=s_ps[:],
                                                    in1=mk[:], op=mul_op)
                            w = wp.tile([JB, IBW], mdt, tag="w")
                            nc.scalar.activation(w[:], u[:], lrelu)
                            tiles.append(w)
                    pend[idx] = tiles
                if idx >= 2:
                    s2, p2 = seq[idx - 2]
                    cnt2 = 2 * s2 + 2
                    y = ytile[s2]
                    vsl = slice(p2 * F, (p2 + 1) * F)
                    tiles = pend.pop(idx - 2)
                    for comp, (sb_va, psl) in enumerate((
                            (sb_var, slice(0, 64)),
                            (sb_vai, slice(64, 128)))):
                        first = (p2 == 0)
                        last = (p2 == cnt2 - 1 and s2 == 0)
                        nc.tensor.matmul(y[psl, :], sb_va[:, vsl],
                                         tiles[comp][:],
                                         start=first, stop=last)
                    if p2 == cnt2 - 1:
                        # correction matmuls close the accumulation group at
                        # slot end (kp/q resident; no slot-start stall)
                        if s2 > 0:
                            msl = slice(s2 * F, (s2 + 1) * F)
                            isl2 = slice(s2 * IBW, (s2 + 1) * IBW)
                            nc.tensor.matmul(y[0:64, :], sb_mcr[:, msl],
                                             sb_qr[:, isl2],
                                             start=False, stop=True)
                            nc.tensor.matmul(y[64:128, :], sb_mci[:, msl],
                                             sb_qi[:, isl2],
                                             start=False, stop=True)
                        # paired output: two slots share one osb tile / DMA
                        t2, half = divmod(s2, 2)
                        if half == 0:
                            y_sb = ytile[('osb', t2)] = osb.tile(
                                [P, 2 * IBW], mdt, tag="ysb", name=f"ysb{t2}")
                        else:
                            y_sb = ytile[('osb', t2)]
                        nc.scalar.copy(y_sb[:, half * IBW:(half + 1) * IBW],
                                       y[:])
                        if half == 1:
                            osl = slice(t2 * 2 * IBW, (t2 + 1) * 2 * IBW)
                            nc.sync.dma_start(out=out[:, osl], in_=y_sb[:])
    nc.compile()
    return nc


def _prep_inputs(Q, K, V, W_att, b_att):
    """Host-side re-layout: per-core in_maps for run_bass_kernel_spmd."""
    Q = np.asarray(Q, dtype=np.float32)
    K = np.asarray(K, dtype=np.float32)
    V = np.asarray(V, dtype=np.float32)
    W_att = np.asarray(W_att, dtype=np.float32)

    Qf = Q.reshape(B, N, P)          # [b, i, f*2+c]
    Kf = K.reshape(B, N, P)
    Vpr = SCALE * (V[..., 0] @ W_att.T)   # [B, N, F]
    Vpi = SCALE * (V[..., 1] @ W_att.T)

    # causal masks for a slot's last two parity j-blocks, per core parity h:
    # diagonal sub-block d = 2k+h of the slot's group of 4
    jj = np.arange(JB)[:, None]
    ii = np.arange(IBW)[None, :]
    mscale = 1.0 / (1.0 - NEG)
    masks = {h: np.stack([mscale * (ii >= jj + JB * (2 * k + h))
                          .astype(np.float32) for k in range(2)])
             for h in (0, 1)}

    if MM_BF16:
        import ml_dtypes
        cvt = lambda a: np.ascontiguousarray(a).astype(ml_dtypes.bfloat16)
    else:
        cvt = lambda a: np.ascontiguousarray(a, dtype=np.float32)

    in_maps = []
    for c in range(NCORES):
        b, h = divmod(c, 2)
        Qmodr = Qf[b].copy()
        Qmodr[:, 1::2] *= -1.0
        Qmodi = np.empty_like(Qf[b])
        Qmodi[:, 0::2] = Qf[b][:, 1::2]
        Qmodi[:, 1::2] = Qf[b][:, 0::2]
        # parity-packed K: [P, NJPAR*JB], position pp holds block J = 2*pp+h
        kp3 = Kf[b].reshape(N // JB, JB, P)[h::2]          # [16, j, p]
        kp = kp3.transpose(2, 0, 1).reshape(P, -1)         # [p, pp*JB+j]
        vr3 = Vpr[b].reshape(N // JB, JB, F)[h::2]         # [16, j, f]
        vi3 = Vpi[b].reshape(N // JB, JB, F)[h::2]
        vpr = vr3.transpose(1, 0, 2).reshape(JB, -1)       # [j, pp*F+f]
        vpi = vi3.transpose(1, 0, 2).reshape(JB, -1)
        # per-slot correction: 0.01 * sum over FULL blocks (pos < cnt-2 = 2s)
        prod_r = np.einsum('bjp,bjf->bpf', kp3, vr3)       # [16, p, f]
        prod_i = np.einsum('bjp,bjf->bpf', kp3, vi3)
        pre_r = np.concatenate(
            [np.zeros((1, P, F), np.float32), np.cumsum(prod_r, axis=0)])
        pre_i = np.concatenate(
            [np.zeros((1, P, F), np.float32), np.cumsum(prod_i, axis=0)])
        mcr = np.concatenate([NEG * pre_r[2 * s] for s in range(NSLOT)], axis=1)
        mci = np.concatenate([NEG * pre_i[2 * s] for s in range(NSLOT)], axis=1)
        in_maps.append({
            "qrT": cvt(Qmodr.T),
            "qiT": cvt(Qmodi.T),
            "kp": cvt(kp),
            "var": cvt((1.0 - NEG) * vpr),
            "vai": cvt((1.0 - NEG) * vpi),
            "mcr": cvt(mcr),
            "mci": cvt(mci),
            "dmask": cvt(masks[h]),
        })
    return in_maps


def _gather(results, b_att):
    b_att = np.asarray(b_att, dtype=np.float32)
    out = np.empty((B, N, F, 2), dtype=np.float32)
    for b in range(B):
        y = (results[2 * b]["out"].astype(np.float32)
             + results[2 * b + 1]["out"].astype(np.float32))  # [128, N]
        out[b, :, :, 0] = y[0:64].T + b_att[None, :]
        out[b, :, :, 1] = y[64:128].T + b_att[None, :]
    return out


def kernel(Q, K, V, W_att, b_att):
    if "nc" not in _CACHE:
        _CACHE["nc"] = _build_nc()
    nc = _CACHE["nc"]
    in_maps = _prep_inputs(Q, K, V, W_att, b_att)
    res = run_bass_kernel_spmd(nc, in_maps, core_ids=list(range(NCORES)))
    return _gather(res.results, b_att)
